# revision 1
# baseline (speedup 1.0000x reference)
"""GPTQ 4-bit dequant + linear (x @ W.T + bias) on 8 Trainium2 NeuronCores.

Problem shapes (hardcoded):
  x       [4, 2048, 4096] f32   -> flattened to [8192, 4096], replicated
  qweight [16384, 512]    i32   (8x 4-bit nibbles per int32 along K)
  qzeros  [16384, 4]      i32
  scales  [16384, 32]     f32
  bias    [16384]         f32
  out     [4, 2048, 16384] f32

Sharding: column-parallel over out_features. Each of the 8 cores gets a
2048-row slab of qweight/qzeros/scales/bias, x replicated; outputs are
concatenated on the host along the feature axis.

Per-core kernel:
  Phase A: dequantize the int4 slab to bf16 W.T [4096, 2048] resident in
           SBUF (DVE nibble extract + per-group (q-z)*s, xbar DMA transpose).
  Phase B: for each 128-token chunk: cast-DMA x to bf16, xbar-transpose to
           [K, t] layout, 32x4 PE matmuls accumulating [128t, 512n] PSUM
           tiles over K, bias folded in as a K=1 matmul, PSUM->SBUF copy
           on ACT, DMA out.
"""
import sys

for _p in ("/opt/trn_rl_repo", "/root/.axon_site/_ro/trn_rl_repo"):
    if _p not in sys.path:
        sys.path.append(_p)

import numpy as np
import concourse.bass as bass
import concourse.mybir as mybir
from concourse import tile, bacc
from concourse.bass_utils import run_bass_kernel_spmd

BF16 = mybir.dt.bfloat16
F32 = mybir.dt.float32
I32 = mybir.dt.int32

B, S, K, N = 4, 2048, 4096, 16384
T = B * S                      # 8192 tokens
NCORES = 8
NS = N // NCORES               # 2048 out features per core
PACK = 8
GS = 128                       # quant group size
G = K // GS                    # 32 groups
TCH = 128                      # tokens per chunk
KC = K // 128                  # 32 k-chunks
MMN = 512                      # matmul moving free dim (one PSUM bank of f32)
NBLK = NS // MMN               # 4

_LSR = mybir.AluOpType.logical_shift_right
_AND = mybir.AluOpType.bitwise_and
_SUB = mybir.AluOpType.subtract
_MUL = mybir.AluOpType.mult


def build(t_total: int = T):
    nt = t_total // TCH
    nc = bacc.Bacc("TRN2", target_bir_lowering=False, debug=False)
    x_d = nc.dram_tensor("x", [t_total, K], F32, kind="ExternalInput")
    qw_d = nc.dram_tensor("qw", [NS, K // PACK], I32, kind="ExternalInput")
    qz_d = nc.dram_tensor("qz", [NS, G // PACK], I32, kind="ExternalInput")
    sc_d = nc.dram_tensor("sc", [NS, G], F32, kind="ExternalInput")
    b_d = nc.dram_tensor("b", [NS], F32, kind="ExternalInput")
    out_d = nc.dram_tensor("out", [t_total, NS], F32, kind="ExternalOutput")

    with tile.TileContext(nc) as tc:
        with (
            tc.tile_pool(name="wtp", bufs=1) as wtpool,
            tc.tile_pool(name="consts", bufs=1) as cpool,
            tc.tile_pool(name="aload", bufs=2) as apool,
            tc.tile_pool(name="anib", bufs=1) as nibpool,
            tc.tile_pool(name="awch", bufs=1) as wchpool,
            tc.tile_pool(name="bx", bufs=2) as bxpool,
            tc.tile_pool(name="bout", bufs=1) as bopool,
            tc.tile_pool(name="ps", bufs=2, space=bass.MemorySpace.PSUM) as pspool,
        ):
            # persistent dequantized W.T: KC tiles of [128 k, NS n] bf16
            wT = [wtpool.tile([128, NS], BF16, name=f"wT{c}") for c in range(KC)]

            bias_t = cpool.tile([1, NS], BF16)
            nc.gpsimd.dma_start(bias_t[:], b_d[:].rearrange("(o n) -> o n", o=1))
            ones_t = cpool.tile([1, TCH], BF16)
            nc.vector.memset(ones_t[:], 1.0)

            # ---- Phase A: dequantize weight slab, n-chunks of 128 rows
            for j in range(NS // 128):
                n0 = j * 128
                qw_t = apool.tile([128, K // PACK], I32)
                nc.sync.dma_start(qw_t[:], qw_d[n0:n0 + 128, :])
                qz_t = apool.tile([128, G // PACK], I32)
                nc.sync.dma_start(qz_t[:], qz_d[n0:n0 + 128, :])
                sc_t = apool.tile([128, G], F32)
                nc.sync.dma_start(sc_t[:], sc_d[n0:n0 + 128, :])

                zi_t = apool.tile([128, G], I32)
                for i in range(PACK):
                    nc.vector.tensor_scalar(
                        out=zi_t[:, i::PACK], in0=qz_t[:],
                        scalar1=4 * i, scalar2=0xF, op0=_LSR, op1=_AND)
                z_t = apool.tile([128, G], F32)
                nc.vector.tensor_copy(z_t[:], zi_t[:])

                nib_t = nibpool.tile([128, K], I32)
                for i in range(PACK):
                    nc.vector.tensor_scalar(
                        out=nib_t[:, i::PACK], in0=qw_t[:],
                        scalar1=4 * i, scalar2=0xF, op0=_LSR, op1=_AND)

                w_t = wchpool.tile([128, K], BF16)
                for g in range(G):
                    nc.vector.tensor_scalar(
                        out=w_t[:, g * GS:(g + 1) * GS],
                        in0=nib_t[:, g * GS:(g + 1) * GS],
                        scalar1=z_t[:, g:g + 1], scalar2=sc_t[:, g:g + 1],
                        op0=_SUB, op1=_MUL)

                for c in range(KC):
                    nc.sync.dma_start_transpose(
                        wT[c][:, n0:n0 + 128], w_t[:, c * 128:(c + 1) * 128])

            # ---- Phase B: stream tokens
            for ti in range(nt):
                t0 = ti * TCH
                xb_t = bxpool.tile([128, K], BF16)
                nc.gpsimd.dma_start(xb_t[:], x_d[t0:t0 + TCH, :])  # f32->bf16 cast DMA
                xT_t = bxpool.tile([128, K], BF16)
                for c in range(KC):
                    nc.sync.dma_start_transpose(
                        xT_t[:, c * 128:(c + 1) * 128], xb_t[:, c * 128:(c + 1) * 128])

                ps = pspool.tile([128, NS], F32)
                for c in range(KC):
                    lhsT = xT_t[:, c * 128:(c + 1) * 128]
                    for nb in range(NBLK):
                        nc.tensor.matmul(
                            ps[:, nb * MMN:(nb + 1) * MMN], lhsT,
                            wT[c][:, nb * MMN:(nb + 1) * MMN],
                            start=(c == 0), stop=False)
                for nb in range(NBLK):
                    nc.tensor.matmul(
                        ps[:, nb * MMN:(nb + 1) * MMN], ones_t[:],
                        bias_t[:, nb * MMN:(nb + 1) * MMN],
                        start=False, stop=True)

                o_t = bopool.tile([128, NS], F32)
                for nb in range(NBLK):
                    nc.scalar.copy(o_t[:, nb * MMN:(nb + 1) * MMN],
                                   ps[:, nb * MMN:(nb + 1) * MMN])
                nc.sync.dma_start(out_d[t0:t0 + TCH, :], o_t[:])

    nc.compile()
    return nc


_nc_cache = {}


def _get_nc(t_total: int = T):
    if t_total not in _nc_cache:
        _nc_cache[t_total] = build(t_total)
    return _nc_cache[t_total]


def kernel(x, qweight, qzeros, scales, bias, trace=False):
    xf = np.ascontiguousarray(x.reshape(T, K).astype(np.float32, copy=False))
    in_maps = []
    for c in range(NCORES):
        sl = slice(c * NS, (c + 1) * NS)
        in_maps.append({
            "x": xf,
            "qw": np.ascontiguousarray(qweight[sl]),
            "qz": np.ascontiguousarray(qzeros[sl]),
            "sc": np.ascontiguousarray(scales[sl]),
            "b": np.ascontiguousarray(bias[sl]),
        })
    nc = _get_nc()
    res = run_bass_kernel_spmd(nc, in_maps, core_ids=list(range(NCORES)),
                               trace=trace)
    out = np.concatenate([r["out"] for r in res.results], axis=1)
    out = out.reshape(B, S, N).astype(np.float32, copy=False)
    if trace:
        return out, res
    return out



# revision 5
# speedup vs baseline: 1.9614x; 1.9614x over previous
"""GPTQ 4-bit dequant + linear (x @ W.T + bias) on 8 Trainium2 NeuronCores.

Problem shapes (hardcoded):
  x       [4, 2048, 4096] f32   -> flattened to [8192, 4096], replicated
  qweight [16384, 512]    i32   (8x 4-bit nibbles per int32 along K)
  qzeros  [16384, 4]      i32
  scales  [16384, 32]     f32
  bias    [16384]         f32
  out     [4, 2048, 16384] f32

Sharding: column-parallel over out_features. Each of the 8 cores gets a
2048-row slab of qweight/qzeros/scales/bias, x replicated; outputs are
concatenated on the host along the feature axis.

Per-core kernel:
  Phase A: dequantize the int4 slab to bf16 W.T [4096, 2048] resident in
           SBUF. Nibble extract on DVE, per-group (q-z)*s split between
           DVE tensor_scalar and ACT activation(Identity, scale, bias),
           one batched xbar transpose per 128-row n-chunk.
  Phase B: per 128-token chunk: SWDGE cast-DMA x to bf16, ONE batched
           xbar transpose to [128k, 32c, 128t], 32x4 PE matmuls
           accumulating [128t, 4x512n] PSUM over k-chunks, DVE
           PSUM+bias -> SBUF add, store on the ACT HWDGE ring.
"""
import sys

for _p in ("/opt/trn_rl_repo", "/root/.axon_site/_ro/trn_rl_repo"):
    if _p not in sys.path:
        sys.path.append(_p)

import numpy as np
import concourse.bass as bass
import concourse.mybir as mybir
from concourse import tile, bacc
from concourse.bass_utils import run_bass_kernel_spmd

BF16 = mybir.dt.bfloat16
F32 = mybir.dt.float32
I32 = mybir.dt.int32

B, S, K, N = 4, 2048, 4096, 16384
T = B * S                      # 8192 tokens
NCORES = 8
NS = N // NCORES               # 2048 out features per core
PACK = 8
GS = 128                       # quant group size
G = K // GS                    # 32 groups == 32 k-chunks
TCH = 128                      # tokens per chunk
KC = K // 128                  # 32 k-chunks
MMN = 512                      # matmul moving free dim (one PSUM bank of f32)
NBLK = NS // MMN               # 4
NCH = NS // 128                # 16 weight n-chunks
HALF = K // 2                  # dequant processed in 2 half-chunks

_LSR = mybir.AluOpType.logical_shift_right
_AND = mybir.AluOpType.bitwise_and
_SUB = mybir.AluOpType.subtract
_MUL = mybir.AluOpType.mult
_ADD = mybir.AluOpType.add
IDENT = mybir.ActivationFunctionType.Identity

# fraction of the 32 per-group dequant ops on ACT (rest on DVE): 6 of every 8
ACT_MOD = 6


def build(t_total: int = T):
    nt = t_total // TCH
    nc = bacc.Bacc("TRN2", target_bir_lowering=False, debug=False)
    x_d = nc.dram_tensor("x", [t_total, K], F32, kind="ExternalInput")
    qw_d = nc.dram_tensor("qw", [NS, K // PACK], I32, kind="ExternalInput")
    qz_d = nc.dram_tensor("qz", [NS, G // PACK], I32, kind="ExternalInput")
    sc_d = nc.dram_tensor("sc", [NS, G], F32, kind="ExternalInput")
    b_d = nc.dram_tensor("b", [NS], F32, kind="ExternalInput")
    out_d = nc.dram_tensor("out", [t_total, NS], F32, kind="ExternalOutput")

    with tile.TileContext(nc) as tc:
        with (
            tc.tile_pool(name="wtp", bufs=1) as wtpool,
            tc.tile_pool(name="consts", bufs=1) as cpool,
            tc.tile_pool(name="aload", bufs=2) as apool,
            tc.tile_pool(name="anib", bufs=2) as nibpool,
            tc.tile_pool(name="awch", bufs=1) as wchpool,
            tc.tile_pool(name="bx", bufs=2) as bxpool,
            tc.tile_pool(name="bxt", bufs=2) as bxtpool,
            tc.tile_pool(name="bout", bufs=1) as bopool,
            tc.tile_pool(name="ps", bufs=2, space=bass.MemorySpace.PSUM) as pspool,
        ):
            # persistent dequantized W.T: [128 kk, 32 c, 2048 n] bf16
            wT = wtpool.tile([128, KC, NS], BF16)

            # bias broadcast to all 128 partitions: [128, 2048] f32
            bias_t = cpool.tile([128, NS], F32)
            b_row = b_d[:].rearrange("(o n) -> o n", o=1)
            b_bcast = bass.AP(tensor=b_row.tensor, offset=b_row.offset,
                              ap=[[0, 128], b_row.ap[1]])
            nc.gpsimd.dma_start(out=bias_t[:], in_=b_bcast)

            # ---- Phase A: dequantize weight slab, n-chunks of 128 rows
            for j in range(NCH):
                n0 = j * 128
                qw_t = apool.tile([128, K // PACK], I32)
                nc.gpsimd.dma_start(qw_t[:], qw_d[n0:n0 + 128, :])
                qz_t = apool.tile([128, G // PACK], I32)
                nc.gpsimd.dma_start(qz_t[:], qz_d[n0:n0 + 128, :])
                sc_t = apool.tile([128, G], F32)
                nc.gpsimd.dma_start(sc_t[:], sc_d[n0:n0 + 128, :])

                zi_t = apool.tile([128, G], I32)
                for i in range(PACK):
                    nc.vector.tensor_scalar(
                        out=zi_t[:, i::PACK], in0=qz_t[:],
                        scalar1=4 * i, scalar2=0xF, op0=_LSR, op1=_AND)
                z_t = apool.tile([128, G], F32)
                nc.vector.tensor_copy(z_t[:], zi_t[:])
                # zs = -z * s  (ACT bias operand)
                zs_t = apool.tile([128, G], F32)
                nc.vector.scalar_tensor_tensor(
                    out=zs_t[:], in0=z_t[:], scalar=-1.0, in1=sc_t[:],
                    op0=_MUL, op1=_MUL)

                w_t = wchpool.tile([128, K], BF16)
                for h in range(2):
                    k0 = h * HALF
                    w0 = k0 // PACK
                    nib_t = nibpool.tile([128, HALF], I32)
                    for i in range(PACK):
                        nc.vector.tensor_scalar(
                            out=nib_t[:, i::PACK],
                            in0=qw_t[:, w0:w0 + HALF // PACK],
                            scalar1=4 * i, scalar2=0xF, op0=_LSR, op1=_AND)
                    for gh in range(G // 2):
                        g = h * (G // 2) + gh
                        if (g % 8) < ACT_MOD:
                            # ACT: out = nib * s + (-z*s)
                            nc.scalar.activation(
                                w_t[:, g * GS:(g + 1) * GS],
                                nib_t[:, gh * GS:(gh + 1) * GS],
                                IDENT, bias=zs_t[:, g:g + 1],
                                scale=sc_t[:, g:g + 1])
                        else:
                            # DVE: out = (nib - z) * s
                            nc.vector.tensor_scalar(
                                out=w_t[:, g * GS:(g + 1) * GS],
                                in0=nib_t[:, gh * GS:(gh + 1) * GS],
                                scalar1=z_t[:, g:g + 1], scalar2=sc_t[:, g:g + 1],
                                op0=_SUB, op1=_MUL)

                # one batched xbar transpose: w_t [128 n, 4096 k]
                #   -> wT[:, :, n0:n0+128]  ([128 kk, 32 c, 128 n])
                nc.sync.dma_start_transpose(wT[:, :, n0:n0 + 128], w_t[:])

            # ---- Phase B: stream tokens
            for ti in range(nt):
                t0 = ti * TCH
                xb_t = bxpool.tile([128, K], BF16)
                nc.gpsimd.dma_start(xb_t[:], x_d[t0:t0 + TCH, :])  # f32->bf16 cast
                xT_t = bxtpool.tile([128, KC, TCH], BF16)
                nc.sync.dma_start_transpose(xT_t[:], xb_t[:])

                ps = pspool.tile([128, NS], F32)
                for c in range(KC):
                    lhsT = xT_t[:, c, :]
                    for nb in range(NBLK):
                        nc.tensor.matmul(
                            ps[:, nb * MMN:(nb + 1) * MMN], lhsT,
                            wT[:, c, nb * MMN:(nb + 1) * MMN],
                            start=(c == 0), stop=(c == KC - 1))

                o_t = bopool.tile([128, NS], F32)
                nc.vector.tensor_tensor(
                    out=o_t[:], in0=ps[:], in1=bias_t[:], op=_ADD)
                nc.scalar.dma_start(out_d[t0:t0 + TCH, :], o_t[:])

    nc.compile()
    return nc


_nc_cache = {}


def _get_nc(t_total: int = T):
    if t_total not in _nc_cache:
        _nc_cache[t_total] = build(t_total)
    return _nc_cache[t_total]


def kernel(x, qweight, qzeros, scales, bias, trace=False, t_total=T):
    xf = np.ascontiguousarray(
        x.reshape(-1, K)[:t_total].astype(np.float32, copy=False))
    in_maps = []
    for c in range(NCORES):
        sl = slice(c * NS, (c + 1) * NS)
        in_maps.append({
            "x": xf,
            "qw": np.ascontiguousarray(qweight[sl]),
            "qz": np.ascontiguousarray(qzeros[sl]),
            "sc": np.ascontiguousarray(scales[sl]),
            "b": np.ascontiguousarray(bias[sl]),
        })
    nc = _get_nc(t_total)
    res = run_bass_kernel_spmd(nc, in_maps, core_ids=list(range(NCORES)),
                               trace=trace)
    out = np.concatenate([r["out"] for r in res.results], axis=1)
    if t_total == T:
        out = out.reshape(B, S, N)
    out = out.astype(np.float32, copy=False)
    if trace:
        return out, res
    return out
